# revision 51
# baseline (speedup 1.0000x reference)
"""2-layer GAT on 8 trn2 NeuronCores.

Strategy: shard dst nodes across 8 cores with degree-balanced grouping
(128 dst nodes per group, ~640 edges each). 3 sequential SPMD bass
kernels; host stages tables / halo scalars between layers (host work is
data staging only — all value compute for the heavy dims is on-device):

  K1: [feat | el | er] = X @ [W1 | W1@Al | W1@Ar]   (bf16 GEMM)
  host: per-(core,part) compacted src feature tables (<32768 rows so a
        single int16 dma_gather covers each group); scatter el[src],
        er[dst] (device-computed) into edge-slot layout; one-hot
        slot->dst matrices.
  K2: per group: gather src feats (random edges) + contiguous self-loop
      column, p=exp(lrelu(el+er)), scale, one-hot matmul aggregation +
      softmax denom, relu, feat2 = h @ [W2|W2@A2].
  K3: same edge phase on layer-2 feats + head-mean epilogue.

Features are stored head-interleaved (d-major: col = d*H+h) so the DVE
per-head broadcast multiplies have stride-1 innermost runs (2.8x faster
than head-major). Weight matrices are permuted on host to compensate.

Edge layout per core: edges grouped by dst group (128 dst nodes), slot
(p, j) = edge j*128+p; last column of each group = self-loop edges
(slot p = node at position p), loaded with one contiguous DMA.
"""
import os
import sys
import numpy as np

sys.path.insert(0, "/opt/trn_rl_repo")

try:
    import antenv
    _ap = os.path.join(os.path.dirname(antenv.__file__), "axon_hooks.py")
    if not os.path.exists(_ap):
        with open(_ap, "w") as _f:
            _f.write(
                "_HOOK = None\n\n"
                "def set_axon_ntff_profile_hook(hook):\n"
                "    global _HOOK\n    _HOOK = hook\n\n"
                "def get_axon_ntff_profile_hook():\n    return _HOOK\n")
except Exception:
    pass

import ml_dtypes
import concourse.bacc as bacc
import concourse.bass as bass
import concourse.mybir as mybir
import concourse.tile as tile
from concourse.bass_utils import run_bass_kernel_spmd

f32 = mybir.dt.float32
f32r = mybir.dt.float32r
bf16 = mybir.dt.bfloat16
i16 = mybir.dt.int16

BF16 = ml_dtypes.bfloat16
NCORES = 8
HEADS = 8
SLOPE = 0.2
BLK = 128            # dst nodes per group
TAB_ROWS = 32768     # rows per compacted src sub-table (int16 idx limit)
PAD_ROW = TAB_ROWS - 1
PAD_EL = -1.0e5

_exec_ns = {"total": 0}


def _ru(x, m):
    return (x + m - 1) // m * m


# ----------------------------------------------------------------------
# host-side graph plan
# ----------------------------------------------------------------------
class Plan:
    pass


def _serpentine(num, nbins):
    i = np.arange(num)
    rows, cols = i // nbins, i % nbins
    return np.where(rows % 2 == 0, cols, nbins - 1 - cols)


def build_plan(src, dst, n):
    src = np.asarray(src, np.int64)
    dst = np.asarray(dst, np.int64)
    pn = n // NCORES
    ngrp = _ru(pn, BLK) // BLK
    pn_pad = ngrp * BLK

    deg = np.bincount(dst, minlength=n)

    order = np.argsort(-deg, kind="stable")
    core_of = np.empty(n, np.int32)
    core_of[order] = _serpentine(n, NCORES)

    grp_of = np.empty(n, np.int32)
    pos_of = np.empty(n, np.int32)
    perm = np.full((NCORES, pn_pad), -1, np.int64)
    for c in range(NCORES):
        nodes_c = np.where(core_of == c)[0]
        o = np.argsort(-deg[nodes_c], kind="stable")
        nodes_s = nodes_c[o]
        g = _serpentine(len(nodes_s), ngrp)
        gsum = np.bincount(g, weights=deg[nodes_s], minlength=ngrp)
        for _ in range(2000):
            gmax, gmin = int(np.argmax(gsum)), int(np.argmin(gsum))
            diff = gsum[gmax] - gsum[gmin]
            if diff <= 2:
                break
            im = np.where(g == gmax)[0]
            il = np.where(g == gmin)[0]
            dm, dl_ = deg[nodes_s[im]], deg[nodes_s[il]]
            bi = im[np.argmax(dm)]
            bj = il[np.argmin(np.abs(dl_ - (deg[nodes_s[bi]] - diff / 2.0)))]
            d = deg[nodes_s[bi]] - deg[nodes_s[bj]]
            if d <= 0:
                break
            gsum[gmax] -= d
            gsum[gmin] += d
            g[bi], g[bj] = gmin, gmax
        ordg = np.argsort(g, kind="stable")
        gg = g[ordg]
        p_arr = np.empty(len(nodes_s), np.int32)
        p_arr[ordg] = np.arange(len(nodes_s)) - np.searchsorted(gg, gg)
        assert p_arr.max() < BLK, "group overflow"
        grp_of[nodes_s] = g
        pos_of[nodes_s] = p_arr
        perm[c, g * BLK + p_arr] = nodes_s

    e_core = core_of[dst]
    e_grp = grp_of[dst]
    e_pos = pos_of[dst]
    # classify self-loop edges (<=1 per node goes to the diag column)
    sel = np.zeros(len(src), bool)
    idxs = np.where(src == dst)[0]
    _, first_pos = np.unique(dst[idxs], return_index=True)
    sel[idxs[first_pos]] = True

    # random-edge counts per (core, group); groups merged in PAIRS
    assert ngrp % 2 == 0
    npair = ngrp // 2
    cnt = np.zeros((NCORES, ngrp), np.int64)
    for c in range(NCORES):
        cnt[c] = np.bincount(e_grp[(e_core == c) & ~sel], minlength=ngrp)
    krand = np.maximum(1, (cnt.max(0) + BLK - 1) // BLK).astype(np.int64)
    # pair u = groups (2u, 2u+1); cols: [rand0 | rand1 | self0 | self1]
    kru = krand[0::2] + krand[1::2]
    ku = kru + 2
    poff = np.concatenate([[0], np.cumsum(ku)])     # col offset per pair
    ksum = int(poff[-1])
    # column offset of each group's random cols / self col
    gcol = np.empty(ngrp, np.int64)                  # first random col
    scol = np.empty(ngrp, np.int64)                  # self col
    for t in range(ngrp):
        u, b = t // 2, t % 2
        gcol[t] = poff[u] + (krand[t - 1] if b else 0)
        scol[t] = poff[u] + kru[u] + b

    slot_src = np.full((NCORES, 128, ksum), -1, np.int64)
    slot_dst = np.full((NCORES, 128, ksum), -1.0, np.float32)
    for c in range(NCORES):
        m = (e_core == c) & ~sel
        es, eg, ep = src[m], e_grp[m], e_pos[m]
        o = np.argsort(eg, kind="stable")
        es, eg, ep = es[o], eg[o], ep[o]
        i_in_g = np.arange(len(eg)) - np.searchsorted(eg, eg)
        col = gcol[eg] + i_in_g // 128
        row = i_in_g % 128
        slot_src[c, row, col] = es
        slot_dst[c, row, col] = ep
        ms = (e_core == c) & sel
        slot_src[c, e_pos[ms], scol[e_grp[ms]]] = src[ms]
        slot_dst[c, e_pos[ms], scol[e_grp[ms]]] = e_pos[ms]

    # parts: contiguous PAIR ranges with <=TAB_ROWS-1 distinct random srcs
    def _rand_cols(u0, u1):
        return np.concatenate(
            [poff[u] + np.arange(kru[u]) for u in range(u0, u1)])

    parts = []
    u0 = 0
    limit = TAB_ROWS - 1
    while u0 < npair:
        u1 = npair
        while True:
            ok = True
            for c in range(NCORES):
                seg = slot_src[c][:, _rand_cols(u0, u1)]
                if len(np.unique(seg[seg >= 0])) > limit:
                    ok = False
                    break
            if ok:
                break
            u1 = u0 + max(1, (u1 - u0) * 3 // 4)
        parts.append((u0, int(u1)))
        u0 = int(u1)
    npart = len(parts)
    part_of_u = np.empty(npair, np.int32)
    for pi, (a, b) in enumerate(parts):
        part_of_u[a:b] = pi

    rows_of = [[np.empty(0, np.int64)] * npart for _ in range(NCORES)]
    kroff = np.concatenate([[0], np.cumsum(kru)])    # gather idx offset/pair
    krsum = int(kroff[-1])
    idx16 = np.full((NCORES, 128, 8 * krsum), PAD_ROW, np.int16)
    for c in range(NCORES):
        for pi, (a, b) in enumerate(parts):
            seg = slot_src[c][:, _rand_cols(a, b)]
            uniq = np.unique(seg[seg >= 0])
            rows_of[c][pi] = uniq
            loc = np.searchsorted(uniq, seg)
            loc[seg < 0] = PAD_ROW
            cbase = 0
            for u in range(a, b):
                k = int(kru[u])
                lt = loc[:, cbase:cbase + k]
                cbase += k
                v = lt.T.reshape(-1)
                w = v.reshape(k * 8, 16).T
                idx16[c, :, 8 * kroff[u]:8 * kroff[u] + 8 * k] = np.tile(w, (8, 1))

    # host-built one-hot matrices (all columns incl self)
    s0 = np.zeros((NCORES, 128, ksum, 128), BF16)
    for c in range(NCORES):
        d_ = slot_dst[c].astype(np.int64)
        m = d_ >= 0
        p_i, c_i = np.where(m)
        s0[c, p_i, c_i, d_[m]] = 1.0

    pl = Plan()
    pl.n, pl.pn, pl.ngrp, pl.pn_pad = n, pn, ngrp, pn_pad
    pl.npair, pl.krand, pl.kru, pl.ku = npair, krand, kru, ku
    pl.poff, pl.kroff = poff, kroff
    pl.gcol, pl.scol = gcol, scol
    pl.ksum, pl.krsum = ksum, krsum
    pl.parts, pl.npart, pl.part_of_u = parts, npart, part_of_u
    pl.perm, pl.rows_of = perm, rows_of
    pl.slot_src, pl.slot_dst, pl.idx16 = slot_src, slot_dst, idx16
    pl.s0 = s0.reshape(NCORES, 128, ksum * 128)
    return pl


def stage_tables(pl, feat, d_pad):
    """feat [n, d] bf16 -> sub-tables [NCORES, npart, TAB_ROWS, d_pad] and
    node-ordered tables [NCORES, pn_pad, d_pad]."""
    d = feat.shape[1]
    tabs = np.zeros((NCORES, pl.npart, TAB_ROWS, d_pad), BF16)
    for c in range(NCORES):
        for pi in range(pl.npart):
            r = pl.rows_of[c][pi]
            tabs[c, pi, :len(r), :d] = feat[r]
    nt = np.zeros((NCORES, pl.pn_pad, d_pad), BF16)
    for c in range(NCORES):
        m = pl.perm[c] >= 0
        nt[c, m, :d] = feat[pl.perm[c][m]]
    return tabs, nt


def stage_z(pl, el, er):
    """z[slot] = el[src] + er[dst]  (pads -> PAD_EL)."""
    out = np.full((NCORES, 128, pl.ksum, HEADS), PAD_EL, np.float32)
    g_of_col = np.empty(pl.ksum, np.int64)
    for t in range(pl.ngrp):
        g_of_col[pl.gcol[t]:pl.gcol[t] + pl.krand[t]] = t
        g_of_col[pl.scol[t]] = t
    for c in range(NCORES):
        s = pl.slot_src[c]
        d = pl.slot_dst[c].astype(np.int64)
        m = s >= 0
        dn = pl.perm[c][(g_of_col[None, :] * BLK + d).clip(0, pl.pn_pad - 1)]
        out[c][m] = el[s[m]] + er[dn[m]]
    return out


# ----------------------------------------------------------------------
# K1: [feat|el|er] = X @ W1ext   (batched blocks)
# ----------------------------------------------------------------------
def build_k1(ngrp, d_in, d_out, BB=None):
    kc = d_in // 128
    if BB is None:
        BB = next(b for b in (7, 4, 2, 1) if ngrp % b == 0)
    nbat = ngrp // BB
    nc = bacc.Bacc()
    xtb = nc.declare_dram_parameter("xtb", [128, ngrp * kc * 128], bf16, isOutput=False)
    wt = nc.declare_dram_parameter("wt", [128, kc * d_out], bf16, isOutput=False)
    f1 = nc.declare_dram_parameter("f1", [ngrp * 128, d_out], bf16, isOutput=True)
    with tile.TileContext(nc) as tc:
        with (
            tc.tile_pool(name="const", bufs=1) as cpool,
            tc.tile_pool(name="sbuf", bufs=3) as pool,
            tc.tile_pool(name="psum", bufs=2, space="PSUM") as psum,
        ):
            wtt = cpool.tile([128, kc, d_out], bf16)
            nc.sync.dma_start(
                out=wtt[:], in_=wt[:].rearrange("p (a d) -> p a d", d=d_out))
            for tb in range(nbat):
                lt = pool.tile([128, BB, kc, 128], bf16, tag="lt")
                nc.sync.dma_start(
                    out=lt[:],
                    in_=xtb[:, tb * BB * kc * 128:(tb + 1) * BB * kc * 128]
                        .rearrange("p (g a b) -> p g a b", a=kc, b=128))
                ft = pool.tile([128, BB, d_out], bf16, tag="ft")
                for b in range(BB):
                    acc = psum.tile([128, 512], f32, tag="acc")
                    acc2 = psum.tile([128, d_out - 512], f32, tag="acc2")
                    for a in range(kc):
                        nc.tensor.matmul(acc[:], lhsT=lt[:, b, a, :],
                                         rhs=wtt[:, a, :512],
                                         start=(a == 0), stop=(a == kc - 1))
                        nc.tensor.matmul(acc2[:], lhsT=lt[:, b, a, :],
                                         rhs=wtt[:, a, 512:],
                                         start=(a == 0), stop=(a == kc - 1))
                    nc.scalar.copy(out=ft[:, b, :512], in_=acc[:])
                    nc.scalar.copy(out=ft[:, b, 512:], in_=acc2[:])
                nc.gpsimd.dma_start(
                    out=f1[tb * BB * 128:(tb + 1) * BB * 128, :]
                        .rearrange("(g p) d -> p g d", p=128),
                    in_=ft[:])
    nc.finalize()
    return nc


# ----------------------------------------------------------------------
# K2/K3 shared edge phase
# ----------------------------------------------------------------------
def edge_phase(nc, tc, pools, pl, d_feat, d_pad, prm, kmax, epilogue,
               asum_in_pad=False, gt_bufs=8, num_bufs=2):
    """Per group: gather random-edge rows + contiguous self column, softmax
    weights, one-hot matmul aggregation. Features are head-interleaved
    (innermost dim = HEADS). epilogue(t, num_ps, as_ps). When asum_in_pad,
    pt is copied into row pad columns [d_feat:d_feat+H] and the softmax
    denominator comes out of the num matmul (as_ps=None; epilogue reads
    num_ps[:, d_feat:d_feat+H])."""
    cpool, pool, spool, psum = pools
    dh = d_feat // HEADS
    nw = d_feat + HEADS if asum_in_pad else d_feat

    idx_t = cpool.tile([128, 8 * pl.krsum], i16)
    nc.sync.dma_start(out=idx_t[:], in_=prm["idx"][:])
    z_t = cpool.tile([128, pl.ksum, HEADS], f32)
    nc.gpsimd.dma_start(
        out=z_t[:], in_=prm["z"][:].rearrange("p (k x) -> p k x", x=HEADS))

    for u in range(pl.npair):
        k = int(pl.ku[u])          # rand0+rand1+2 self cols
        kr = int(pl.kru[u])
        o = int(pl.poff[u])
        orr = int(pl.kroff[u])
        pi = int(pl.part_of_u[u])
        gt = pool.tile([128, kmax, d_pad], bf16, tag="gt", bufs=gt_bufs)
        for c0 in range(0, kr, 8):      # <=1024 idxs per gather call
            w = min(8, kr - c0)
            nc.gpsimd.dma_gather(
                out_ap=gt[:, c0:c0 + w, :],
                in_ap=prm["tabs"][pi][:],
                idxs_ap=idx_t[:, 8 * (orr + c0):8 * (orr + c0 + w)],
                num_idxs=128 * w, num_idxs_reg=128 * w, elem_size=d_pad,
                queue_num=u % 4, single_packet=False,
            )
        nc.sync.dma_start(
            out=gt[:, kr:kr + 2, :],
            in_=prm["nt"][2 * u * 128:(2 * u + 2) * 128, :]
                .rearrange("(g p) d -> p g d", p=128))
        s0 = spool.tile([128, kmax, 128], bf16, tag="s0", bufs=4)
        nc.sync.dma_start(
            out=s0[:, :k, :],
            in_=prm["s0"][:, o * 128:(o + k) * 128]
                .rearrange("p (k c) -> p k c", c=128))
        zl = spool.tile([128, kmax, HEADS], f32, tag="zl")
        nc.vector.scalar_tensor_tensor(out=zl[:, :k, :],
                                       in0=z_t[:, o:o + k, :],
                                       scalar=SLOPE, in1=z_t[:, o:o + k, :],
                                       op0=mybir.AluOpType.mult,
                                       op1=mybir.AluOpType.max)
        pt = spool.tile([128, kmax, HEADS], bf16, tag="pt")
        nc.scalar.activation(out=pt[:, :k, :], in_=zl[:, :k, :],
                             func=mybir.ActivationFunctionType.Exp)
        # interleaved per-head scale: innermost dim (HEADS) has real stride
        gv = gt[:, :k, :d_feat].rearrange("p k (d h) -> p k d h", h=HEADS)
        nc.vector.tensor_mul(
            out=gv, in0=gv,
            in1=pt[:, :k, None, :].to_broadcast([128, k, dh, HEADS]))
        if asum_in_pad:
            nc.vector.tensor_copy(out=gt[:, :k, d_feat:d_feat + HEADS],
                                  in_=pt[:, :k, :])
        for b in range(2):
            t = 2 * u + b
            kb = int(pl.krand[t])
            j0 = int(pl.gcol[t] - o)
            js = list(range(j0, j0 + kb)) + [kr + b]
            num_ps = psum.tile([128, nw], f32, tag="num", name="num",
                               bufs=num_bufs)
            as_ps = None
            if asum_in_pad:
                for i, j in enumerate(js):
                    nc.tensor.matmul(num_ps[:], lhsT=s0[:, j, :],
                                     rhs=gt[:, j, :nw],
                                     start=(i == 0), stop=(i == len(js) - 1))
            else:
                as_ps = psum.tile([128, HEADS], f32, tag="asum", name="asum",
                                  bufs=min(num_bufs, 2))
                for i, j in enumerate(js):
                    nc.tensor.matmul(num_ps[:], lhsT=s0[:, j, :],
                                     rhs=gt[:, j, :d_feat],
                                     start=(i == 0), stop=(i == len(js) - 1))
                    nc.tensor.matmul(as_ps[:], lhsT=s0[:, j, :],
                                     rhs=pt[:, j, :],
                                     start=(i == 0), stop=(i == len(js) - 1))
            epilogue(t, num_ps, as_ps)


def _declare_edge_params(nc, pl, d_pad):
    prm = {"tabs": [
        nc.declare_dram_parameter(f"tab{pi}", [TAB_ROWS, d_pad], bf16,
                                  isOutput=False)
        for pi in range(pl.npart)]}
    prm["nt"] = nc.declare_dram_parameter("nt", [pl.pn_pad, d_pad], bf16, isOutput=False)
    prm["idx"] = nc.declare_dram_parameter("idx", [128, 8 * pl.krsum], i16, isOutput=False)
    prm["s0"] = nc.declare_dram_parameter("s0", [128, pl.ksum * 128], bf16, isOutput=False)
    prm["z"] = nc.declare_dram_parameter("z", [128, pl.ksum * HEADS], f32, isOutput=False)
    return prm


def build_k2(pl, d1, d2e, d_pad):
    kmax = int(pl.ku.max())
    kc1 = d1 // 128
    dh1 = d1 // HEADS
    nc = bacc.Bacc(num_swdge_queues=4)
    prm = _declare_edge_params(nc, pl, d_pad)
    wt2 = nc.declare_dram_parameter("wt2", [128, kc1 * d2e], bf16, isOutput=False)
    identp = nc.declare_dram_parameter("identp", [128, 128], bf16, isOutput=False)
    f2 = nc.declare_dram_parameter("f2", [pl.pn_pad, d2e], bf16, isOutput=True)
    with tile.TileContext(nc) as tc:
        with (
            tc.tile_pool(name="const", bufs=1) as cpool,
            tc.tile_pool(name="sbuf", bufs=2) as pool,
            tc.tile_pool(name="small", bufs=3) as spool,
            tc.tile_pool(name="psum", bufs=2, space="PSUM") as psum,
        ):
            wt2t = cpool.tile([128, kc1, d2e], bf16)
            nc.sync.dma_start(
                out=wt2t[:], in_=wt2[:].rearrange("p (a d) -> p a d", d=d2e))
            ident = cpool.tile([128, 128], bf16)
            nc.sync.dma_start(out=ident[:], in_=identp[:])

            def epilogue(t, num_ps, as_ps):
                rec = spool.tile([128, HEADS], bf16, tag="rec")
                with nc.allow_low_precision(reason="softmax denom to bf16"):
                    nc.vector.reciprocal(out=rec[:], in_=as_ps[:])
                h1 = spool.tile([128, d1], bf16, tag="h1", bufs=2)
                nc.scalar.activation(out=h1[:], in_=num_ps[:],
                                     func=mybir.ActivationFunctionType.Relu)
                h = spool.tile([128, d1], bf16, tag="h", bufs=2)
                nc.vector.tensor_mul(
                    out=h[:].rearrange("p (d h) -> p d h", h=HEADS),
                    in0=h1[:].rearrange("p (d h) -> p d h", h=HEADS),
                    in1=rec[:, None, :].to_broadcast([128, dh1, HEADS]))
                ht_ps = psum.tile([128, kc1, 128], bf16, tag="ht", bufs=1)
                for a in range(kc1):
                    nc.tensor.transpose(out=ht_ps[:, a, :],
                                        in_=h[:, a * 128:(a + 1) * 128],
                                        identity=ident[:])
                ht = spool.tile([128, kc1, 128], bf16, tag="hts", bufs=2)
                nc.scalar.copy(out=ht[:], in_=ht_ps[:])
                f2_ps = psum.tile([128, d2e], f32, tag="f2")
                for a in range(kc1):
                    nc.tensor.matmul(f2_ps[:], lhsT=ht[:, a, :], rhs=wt2t[:, a, :],
                                     start=(a == 0), stop=(a == kc1 - 1))
                f2s = spool.tile([128, d2e], bf16, tag="f2s", bufs=2)
                nc.scalar.copy(out=f2s[:], in_=f2_ps[:])
                nc.sync.dma_start(out=f2[t * 128:(t + 1) * 128, :], in_=f2s[:])

            edge_phase(nc, tc, (cpool, pool, spool, psum), pl, d1, d_pad,
                       prm, kmax, epilogue, gt_bufs=5, num_bufs=3)
    nc.finalize()
    return nc


def build_k3(pl, d2, d_pad, ncls, OUT_B=7):
    kmax = int(pl.ku.max())
    ngrp = pl.ngrp
    nc = bacc.Bacc(num_swdge_queues=4)
    prm = _declare_edge_params(nc, pl, d_pad)
    out_o = nc.declare_dram_parameter("out", [128, pl.ngrp * ncls], f32, isOutput=True)
    with tile.TileContext(nc) as tc:
        with (
            tc.tile_pool(name="const", bufs=1) as cpool,
            tc.tile_pool(name="sbuf", bufs=2) as pool,
            tc.tile_pool(name="small", bufs=3) as spool,
            tc.tile_pool(name="psum", bufs=2, space="PSUM") as psum,
        ):
            ob = {}

            def epilogue(t, num_ps, as_ps):
                rec = spool.tile([128, HEADS], f32, tag="rec")
                nc.vector.reciprocal(out=rec[:], in_=as_ps[:])
                if t % OUT_B == 0:
                    ob["tile"] = spool.tile([128, OUT_B, ncls], f32, tag="ot",
                                            bufs=2, name="ot")
                tmp = spool.tile([128, ncls, HEADS], f32, tag="tmp")
                nc.vector.tensor_mul(
                    out=tmp[:],
                    in0=num_ps[:].rearrange("p (c h) -> p c h", h=HEADS),
                    in1=rec[:, None, :].to_broadcast([128, ncls, HEADS]))
                bi = t % OUT_B
                nc.vector.reduce_sum(out=ob["tile"][:, bi, :], in_=tmp[:],
                                     axis=mybir.AxisListType.X)
                if bi == OUT_B - 1 or t == ngrp - 1:
                    t0 = t - bi
                    nc.sync.dma_start(
                        out=out_o[:, t0 * ncls:(t + 1) * ncls],
                        in_=ob["tile"][:, :bi + 1, :]
                            .rearrange("p g c -> p (g c)"))

            edge_phase(nc, tc, (cpool, pool, spool, psum), pl, d2, d_pad,
                       prm, kmax, epilogue)
    nc.finalize()
    return nc


# ----------------------------------------------------------------------
# orchestration
# ----------------------------------------------------------------------
def _run(nc, in_maps, label):
    try:
        res = run_bass_kernel_spmd(nc, in_maps, core_ids=list(range(NCORES)),
                                   trace=True)
    except (ImportError, ModuleNotFoundError):
        res = run_bass_kernel_spmd(nc, in_maps, core_ids=list(range(NCORES)),
                                   trace=False)
    if res.exec_time_ns:
        _exec_ns[label] = res.exec_time_ns
        _exec_ns["total"] += res.exec_time_ns
    return res.results


def _ext(W, al, ar, dh, il_out):
    """[W_perm | W@Al | W@Ar]: W cols permuted to head-interleaved via il_out.
    el/er projections use the ORIGINAL head-major W."""
    d_in, d_out = W.shape
    A = np.zeros((d_out, 2 * HEADS), np.float64)
    for h in range(HEADS):
        A[h * dh:(h + 1) * dh, h] = al[h]
        A[h * dh:(h + 1) * dh, HEADS + h] = ar[h]
    Wp = W[:, il_out]
    return np.concatenate([Wp, W.astype(np.float64) @ A], 1).astype(np.float32)


def _il(d_out):
    """cols: interleaved position i = d*H+h  <- head-major index h*dh+d."""
    dh = d_out // HEADS
    src = np.empty(d_out, np.int64)
    for h in range(HEADS):
        for d in range(dh):
            src[d * HEADS + h] = h * dh + d
    return src


def kernel(features, W1, al1, ar1, b1, W2, al2, ar2, b2, src, dst):
    features = np.asarray(features, np.float32)
    n, d_in = features.shape
    d1 = np.asarray(W1).shape[1]            # 512
    d2 = np.asarray(W2).shape[1]            # 320
    ncls = d2 // HEADS
    assert not np.any(np.asarray(b1)) and not np.any(np.asarray(b2)), \
        "nonzero bias path not implemented"

    pl = build_plan(src, dst, n)
    d1e = d1 + 2 * HEADS                    # 528
    d2e = d2 + 2 * HEADS                    # 336
    d1_pad = _ru(d1, 128)                   # 512
    d2_pad = _ru(d2, 128)                   # 384
    il1 = _il(d1)                           # layer-1 interleave (d-major)
    il2 = _il(d2)                           # layer-2 interleave (c-major)

    W1e = _ext(np.asarray(W1, np.float32), np.asarray(al1, np.float32),
               np.asarray(ar1, np.float32), d1 // HEADS, il1)
    # W2 rows must be in interleaved layer-1 order (h is interleaved)
    W2r = np.asarray(W2, np.float32)[il1, :]
    W2e = _ext(W2r, np.asarray(al2, np.float32),
               np.asarray(ar2, np.float32), ncls, il2)
    W2e[:, :d2] *= 1.0 / HEADS      # fold head-mean into feat2 (exact in bf16)
    # NOTE: el2/er2 projections in _ext used W2r with head-major col order
    # of layer-2 — requires al2/ar2 applied against head-major cols of W2r,
    # which is what _ext does (A indexes cols h*ncls+c of W2r). Correct.

    kc = d_in // 128
    kc1 = d1 // 128
    ident_np = np.eye(128, dtype=np.float32).astype(BF16)

    # ---- K1 ----
    wt_np = np.ascontiguousarray(
        W1e.reshape(kc, 128, d1e).transpose(1, 0, 2).reshape(128, kc * d1e)
    ).astype(BF16)
    k1 = build_k1(pl.ngrp, d_in, d1e)
    in_maps = []
    for c in range(NCORES):
        Xp = np.zeros((pl.pn_pad, d_in), np.float32)
        m = pl.perm[c] >= 0
        Xp[m] = features[pl.perm[c][m]]
        xtb = (Xp.reshape(pl.ngrp, 128, kc, 128)
               .transpose(3, 0, 2, 1).reshape(128, pl.ngrp * kc * 128))
        in_maps.append({"xtb": np.ascontiguousarray(xtb).astype(BF16),
                        "wt": wt_np})
    r1 = _run(k1, in_maps, "k1")

    # ---- host: stage layer-1 tables + halo scalars ----
    f1g = np.zeros((n, d1e), np.float32)
    for c in range(NCORES):
        m = pl.perm[c] >= 0
        f1g[pl.perm[c][m]] = np.asarray(r1[c]["f1"], np.float32)[m]
    tabs1, nt1 = stage_tables(pl, f1g[:, :d1].astype(BF16), d1_pad)
    z1 = stage_z(pl, f1g[:, d1:d1 + HEADS], f1g[:, d1 + HEADS:])

    # ---- K2 ----
    wt2_np = np.ascontiguousarray(
        W2e.reshape(kc1, 128, d2e).transpose(1, 0, 2).reshape(128, kc1 * d2e)
    ).astype(BF16)
    k2 = build_k2(pl, d1, d2e, d1_pad)
    in_maps = []
    for c in range(NCORES):
        im = {f"tab{pi}": tabs1[c, pi] for pi in range(pl.npart)}
        im.update({
            "nt": nt1[c], "idx": pl.idx16[c], "s0": pl.s0[c],
            "z": np.ascontiguousarray(z1[c].reshape(128, -1)),
            "wt2": wt2_np, "identp": ident_np})
        in_maps.append(im)
    r2 = _run(k2, in_maps, "k2")

    # ---- host: stage layer-2 tables + halo scalars ----
    f2g = np.zeros((n, d2e), np.float32)
    for c in range(NCORES):
        m = pl.perm[c] >= 0
        f2g[pl.perm[c][m]] = np.asarray(r2[c]["f2"], np.float32)[m]
    tabs2, nt2 = stage_tables(pl, f2g[:, :d2].astype(BF16), d2_pad)
    z2 = stage_z(pl, f2g[:, d2:d2 + HEADS], f2g[:, d2 + HEADS:])

    # ---- K3 ----
    k3 = build_k3(pl, d2, d2_pad, ncls)
    in_maps = []
    for c in range(NCORES):
        im = {f"tab{pi}": tabs2[c, pi] for pi in range(pl.npart)}
        im.update({
            "nt": nt2[c], "idx": pl.idx16[c], "s0": pl.s0[c],
            "z": np.ascontiguousarray(z2[c].reshape(128, -1))})
        in_maps.append(im)
    r3 = _run(k3, in_maps, "k3")

    out = np.zeros((n, ncls), np.float32)
    for c in range(NCORES):
        # out param layout [128 pos, ngrp*ncls] -> [ngrp*128 node, ncls]
        ov = (np.asarray(r3[c]["out"], np.float32)
              .reshape(128, pl.ngrp, ncls).transpose(1, 0, 2)
              .reshape(pl.pn_pad, ncls))
        m = pl.perm[c] >= 0
        out[pl.perm[c][m]] = ov[m]
    return out
